# revision 1
# baseline (speedup 1.0000x reference)
"""Trainium2 Bass kernel for bidirectional RAFT-style correlation pyramid lookup
(AMT BidirCorrBlock + _corr_scale_lookup, B=1, D=128, H=60, W=108, r=3, L=4).

Strategy (8 NeuronCores, SPMD, no collectives):
  - Shard the 6480 query pixels across cores (810 each, padded to 896 = 7x128).
  - Per 128-query chunk and direction:
      * PE matmul (bf16): corr level0 = f_sliceT @ f_other  [128 x 6480]
      * levels 1-3 via pooled-feature matmuls (exact by linearity of pooling)
      * drain PSUM -> SBUF bf16 with 1/sqrt(D) (and 4^-l pool) scaling, into
        an unpadded query-major layout; ONE DMA to a DRAM pyramid (4 reused
        buffers, margins zeroed once)
      * per query one indirect-DMA band gather per (level, dir): a contiguous
        7*wl+8 run that covers the whole 8x8 bilinear patch (the dynamic
        row/col offset is absorbed into the band start)
      * combined row*col validity mask, separable bilinear lerp ->
        [128, 392] output tile
  - Gather offsets / masks / lerp weights for ALL chunks are computed once
    up front on DVE from flows/embt.
  - Host reassembles [1, 396, 60, 108] (flow passthrough channels appended).
"""

import sys

import numpy as np

sys.path.insert(0, "/opt/trn_rl_repo")

import concourse.bass as bass
import concourse.bacc as bacc
import concourse.mybir as mybir
from concourse.bass import IndirectOffsetOnAxis
from concourse.tile import TileContext

F32 = mybir.dt.float32
BF16 = mybir.dt.bfloat16
I32 = mybir.dt.int32
OP = mybir.AluOpType

# problem geometry
H, W = 60, 108
N = H * W            # 6480
D = 128
NCORES = 8
NQ = N // NCORES     # 810 queries per core
NCH = 7              # chunks per core
CH = 128             # queries per chunk
NQP = NCH * CH       # 896 padded
NL = 4

HL = [60, 30, 15, 7]
WL = [108, 54, 27, 13]
PW = list(WL)                      # rows stored unpadded; col-OOB handled by
                                   # the combined row*col validity mask
LSZ = [HL[l] * PW[l] for l in range(NL)]
LOFF = [0, LSZ[0], LSZ[0] + LSZ[1], LSZ[0] + LSZ[1] + LSZ[2]]
QSTRIDE = sum(LSZ)                 # 10388 elems per query (all 4 levels)
# query-major pyramid: [head margin][q0: L0|L1|L2|L3][q1: ...]...[tail margin]
HEAD_M = 1024                      # >= 8*PW[0] = 992
TAIL_M = 1024
PYR_TOT = HEAD_M + CH * QSTRIDE + TAIL_M

INV_SQRT_D = float(np.float32(1.0) / np.sqrt(np.float32(D)))
MAGIC = float(3 * 2 ** 22)         # x+MAGIC stays in [2^23, 2^24): ulp=1

# const table column layout (cst tensor [128, CW])
_c = 0
def _take(n):
    global _c
    s = _c
    _c += n
    return s
C_LVS = _take(112)    # (c2, ch7, d2, l4): 2^-l
C_CLIM = _take(112)   # (c2, ch7, d2, l4): wl[l] (c==0) else hl[l]
C_ACON = _take(448)   # (ch, d, l, a): a
C_HLM1 = _take(448)   # (ch, d, l, a): hl[l] - 1
C_PW = _take(56)      # (ch, d, l): pw[l]
C_QB = _take(56)      # (ch, d, l): HEAD_M + p*QSTRIDE + LOFF[l]
C_WLM1 = _take(448)   # (ch, d, l, b): wl[l] - 1
CW = _c

AUXW = NCH * 6 + 1    # flows (c,ch,d) 28 | w/h (c,ch) 14 | embt


def host_consts():
    cst = np.zeros((128, CW), np.float32)
    p = np.arange(128)
    for ch in range(NCH):
        for d in range(2):
            for l in range(NL):
                for c in range(2):
                    j = ((c * NCH + ch) * 2 + d) * NL + l
                    cst[:, C_LVS + j] = 0.5 ** l
                    cst[:, C_CLIM + j] = WL[l] if c == 0 else HL[l]
                k = (ch * 2 + d) * NL + l
                cst[:, C_PW + k] = PW[l]
                cst[:, C_QB + k] = HEAD_M + p * QSTRIDE + LOFF[l]
                for a in range(8):
                    cst[:, C_ACON + k * 8 + a] = a
                    cst[:, C_HLM1 + k * 8 + a] = HL[l] - 1
                    cst[:, C_WLM1 + k * 8 + a] = WL[l] - 1
    return cst


def build_nc():
    nc = bacc.Bacc()

    f0 = nc.declare_dram_parameter("fmap0", [D, N], BF16, isOutput=False)
    f1 = nc.declare_dram_parameter("fmap1", [D, N], BF16, isOutput=False)
    f0q = nc.declare_dram_parameter("f0q", [D, NQP], BF16, isOutput=False)
    f1q = nc.declare_dram_parameter("f1q", [D, NQP], BF16, isOutput=False)
    aux = nc.declare_dram_parameter("aux", [128, AUXW], F32, isOutput=False)
    cst = nc.declare_dram_parameter("cst", [128, CW], F32, isOutput=False)
    out = nc.declare_dram_parameter("out", [NQP, 392], F32, isOutput=True)

    with TileContext(nc) as tc:
        cpool = tc.alloc_tile_pool(name="cpool", bufs=1)
        dpool = tc.alloc_tile_pool(name="dpool", bufs=1, space="DRAM")
        ppool = tc.alloc_tile_pool(name="ppool", bufs=4, space="PSUM")
        gpool = tc.alloc_tile_pool(name="gpool", bufs=2)   # bands
        bpool = tc.alloc_tile_pool(name="bpool", bufs=2)   # blend scratch

        # ---- load inputs (cast fmaps to bf16 during DMA) ----
        f0s = cpool.tile([D, N], BF16, name="f0s")
        f1s = cpool.tile([D, N], BF16, name="f1s")
        f0qs = cpool.tile([D, NQP], BF16, name="f0qs")
        f1qs = cpool.tile([D, NQP], BF16, name="f1qs")
        # order by first use: chunk0/dir0 needs f0qs (lhsT) + f1s (rhs)
        nc.scalar.dma_start(out=f0qs[:], in_=f0q[:])
        nc.scalar.dma_start(out=f1s[:], in_=f1[:])
        nc.scalar.dma_start(out=f1qs[:], in_=f1q[:])
        nc.scalar.dma_start(out=f0s[:], in_=f0[:])
        auxs = cpool.tile([128, AUXW], F32, name="auxs")
        csts = cpool.tile([128, CW], F32, name="csts")
        nc.sync.dma_start(out=auxs[:], in_=aux[:])
        nc.sync.dma_start(out=csts[:], in_=cst[:])

        # ---- tscale = [1/embt, 1/(1-embt)] ----
        tsc = cpool.tile([128, 2], F32, name="tsc")
        trc = cpool.tile([128, 2], F32, name="trc")
        ecol = NCH * 6
        nc.vector.tensor_scalar(
            out=trc[:, 0:1], in0=auxs[:, ecol:ecol + 1],
            scalar1=1.0, scalar2=None, op0=OP.mult)
        nc.vector.tensor_scalar(
            out=trc[:, 1:2], in0=auxs[:, ecol:ecol + 1],
            scalar1=-1.0, scalar2=1.0, op0=OP.mult, op1=OP.add)
        nc.vector.reciprocal(out=tsc[:], in_=trc[:])

        # ---- tables for ALL chunks at once ----
        # xy [128, (c2)(ch7)(d2)] = wh + flow*tsc
        xy = cpool.tile([128, 28], F32, name="xy")
        xyv = xy[:].rearrange("p (c h d) -> p c h d", c=2, h=NCH)
        nc.vector.tensor_tensor(
            out=xyv,
            in0=auxs[:, 0:28].rearrange("p (c h d) -> p c h d", c=2, h=NCH),
            in1=tsc[:].unsqueeze(1).unsqueeze(1).broadcast_to([128, 2, NCH, 2]),
            op=OP.mult)
        nc.vector.tensor_tensor(
            out=xyv, in0=xyv,
            in1=auxs[:, 28:42].rearrange("p (c h) -> p c h", c=2).unsqueeze(
                3).broadcast_to([128, 2, NCH, 2]),
            op=OP.add)
        # xyl [128, 112] = (c, ch, d, l)
        xyl = cpool.tile([128, 112], F32, name="xyl")
        nc.vector.tensor_tensor(
            out=xyl[:].rearrange("p (g l) -> p g l", l=NL),
            in0=xy[:].unsqueeze(2).broadcast_to([128, 28, NL]),
            in1=csts[:, C_LVS:C_LVS + 112].rearrange("p (g l) -> p g l", l=NL),
            op=OP.mult)
        # floor + frac + window start (+ clamps)
        rr = cpool.tile([128, 112], F32, name="rr")
        adj = cpool.tile([128, 112], F32, name="adj")
        wfrac = cpool.tile([128, 112], F32, name="wfrac")
        stc = cpool.tile([128, 112], F32, name="stc")
        nc.vector.tensor_scalar(
            out=rr[:], in0=xyl[:], scalar1=MAGIC, scalar2=None, op0=OP.add)
        nc.vector.tensor_scalar(
            out=rr[:], in0=rr[:], scalar1=-MAGIC, scalar2=None, op0=OP.add)
        nc.vector.tensor_tensor(out=adj[:], in0=rr[:], in1=xyl[:], op=OP.is_gt)
        nc.vector.tensor_tensor(out=rr[:], in0=rr[:], in1=adj[:], op=OP.subtract)
        nc.vector.tensor_tensor(out=wfrac[:], in0=xyl[:], in1=rr[:],
                                op=OP.subtract)
        # st = max(floor - 3, -8) ; stc = min(st, clim)
        nc.vector.tensor_scalar(
            out=stc[:], in0=rr[:], scalar1=-3.0, scalar2=-8.0,
            op0=OP.add, op1=OP.max)
        nc.vector.tensor_tensor(
            out=stc[:], in0=stc[:], in1=csts[:, C_CLIM:C_CLIM + 112], op=OP.min)
        stx = stc[:, 0:56]            # (ch, d, l)
        sty = stc[:, 56:112]
        # row validity mask [128, 448] = (ch, d, l, a)
        trow = cpool.tile([128, 448], F32, name="trow")
        m1 = cpool.tile([128, 448], F32, name="m1")
        maskb = cpool.tile([128, 448], F32, name="maskb")
        # st_y (unclamped) = rr_y - 3: recompute from rr to avoid the -8 clamp
        # interfering?  clamp at -8 keeps mask correct: rows -8..-1 are all
        # invalid anyway and a<=7 => st_y+a <= -1 < 0.  min-clamp at hl keeps
        # t >= hl > hl-1 invalid.  So use stc directly.
        nc.vector.tensor_tensor(
            out=trow[:].rearrange("p (g a) -> p g a", a=8),
            in0=sty.unsqueeze(2).broadcast_to([128, 56, 8]),
            in1=csts[:, C_ACON:C_ACON + 448].rearrange("p (g a) -> p g a", a=8),
            op=OP.add)
        nc.vector.tensor_scalar(
            out=m1[:], in0=trow[:], scalar1=0.0, scalar2=None, op0=OP.is_ge)
        nc.vector.tensor_tensor(
            out=maskb[:], in0=trow[:], in1=csts[:, C_HLM1:C_HLM1 + 448],
            op=OP.is_le)
        nc.vector.tensor_tensor(out=maskb[:], in0=maskb[:], in1=m1[:], op=OP.mult)
        # col validity mask (reuse trow/m1 scratch), then combined row*col mask
        colm = cpool.tile([128, 448], F32, name="colm")
        nc.vector.tensor_tensor(
            out=trow[:].rearrange("p (g a) -> p g a", a=8),
            in0=stx.unsqueeze(2).broadcast_to([128, 56, 8]),
            in1=csts[:, C_ACON:C_ACON + 448].rearrange("p (g a) -> p g a", a=8),
            op=OP.add)
        nc.vector.tensor_scalar(
            out=m1[:], in0=trow[:], scalar1=0.0, scalar2=None, op0=OP.is_ge)
        nc.vector.tensor_tensor(
            out=colm[:], in0=trow[:], in1=csts[:, C_WLM1:C_WLM1 + 448],
            op=OP.is_le)
        nc.vector.tensor_tensor(out=colm[:], in0=colm[:], in1=m1[:], op=OP.mult)
        cmb = cpool.tile([128, 3584], BF16, name="cmb")
        nc.vector.tensor_tensor(
            out=cmb[:].rearrange("p (g a b) -> p g a b", g=56, a=8),
            in0=maskb[:].rearrange("p (g a) -> p g a", a=8).unsqueeze(
                3).broadcast_to([128, 56, 8, 8]),
            in1=colm[:].rearrange("p (g b) -> p g b", b=8).unsqueeze(
                2).broadcast_to([128, 56, 8, 8]),
            op=OP.mult)
        # band start offsets [128, 56] (ch, d, l)
        offf = cpool.tile([128, 56], F32, name="offf")
        offs = cpool.tile([128, 56], I32, name="offs")
        nc.vector.tensor_tensor(
            out=offf[:], in0=sty, in1=csts[:, C_PW:C_PW + 56], op=OP.mult)
        nc.vector.tensor_tensor(out=offf[:], in0=offf[:], in1=stx, op=OP.add)
        nc.vector.tensor_tensor(
            out=offf[:], in0=offf[:], in1=csts[:, C_QB:C_QB + 56], op=OP.add)
        nc.vector.tensor_copy(out=offs[:], in_=offf[:])
        # lerp weights: wx expanded over a -> [128, 448]; wy used directly
        wx = wfrac[:, 0:56]
        wy = wfrac[:, 56:112]
        wxe = cpool.tile([128, 448], F32, name="wxe")
        nc.vector.tensor_copy(
            out=wxe[:].rearrange("p (g a) -> p g a", a=8),
            in_=wx.unsqueeze(2).broadcast_to([128, 56, 8]))

        # ---- pooled feature pyramids (unscaled sums; 4^-l folded into drain) ----
        def make_pools(src, tag):
            tiles = []
            hw = [(H, W), (30, 54), (15, 27)]
            cur = src[:].rearrange("p (h w) -> p h w", h=H)
            for li, (hs, ws) in enumerate(hw):
                ho, wo = hs // 2, ws // 2
                t = cpool.tile([128, ho * wo], BF16, name=f"g{li+1}{tag}")
                tv = t[:].rearrange("p (h w) -> p h w", h=ho)
                ev = cur[:, 0:2 * ho, 0:2 * wo]
                ev = ev.rearrange("p (h a) (w b) -> p h a w b", a=2, b=2)
                tmp1 = bpool.tile([128, ho * wo], F32, name=f"pool_t1_{tag}{li}")
                tmp2 = bpool.tile([128, ho * wo], F32, name=f"pool_t2_{tag}{li}")
                t1v = tmp1[:].rearrange("p (h w) -> p h w", h=ho)
                t2v = tmp2[:].rearrange("p (h w) -> p h w", h=ho)
                nc.gpsimd.tensor_tensor(
                    out=t1v, in0=ev[:, :, 0, :, 0], in1=ev[:, :, 0, :, 1], op=OP.add)
                nc.gpsimd.tensor_tensor(
                    out=t2v, in0=ev[:, :, 1, :, 0], in1=ev[:, :, 1, :, 1], op=OP.add)
                nc.gpsimd.tensor_tensor(out=tv, in0=t1v, in1=t2v, op=OP.add)
                tiles.append(t)
                cur = t[:].rearrange("p (h w) -> p h w", h=ho)
            return tiles

        gf = make_pools(f1s, "f")
        gb = make_pools(f0s, "b")

        # ---- persistent chunk tiles (ping/pong), pads pre-zeroed ----
        lt = []
        _frees = []
        for i in range(2):
            t, _f = tc.tile([128, QSTRIDE], BF16, name=f"lt{i}")
            lt.append(t)
            _frees.append(_f)
        zsrc, _fz = tc.tile([1, HEAD_M], BF16, name="zsrc")
        _frees.append(_fz)
        nc.gpsimd.memset(zsrc[:], 0.0)
        pyrs = []
        for i in range(6):
            pt = dpool.tile([PYR_TOT, 1], BF16, name=f"pyr{i}")
            mv = bass.AP(pt[:, 0].tensor, pt[:, 0].offset,
                         [[PYR_TOT - TAIL_M, 2], [1, TAIL_M]])
            nc.sync.dma_start(
                out=mv,
                in_=zsrc[:].unsqueeze(1).broadcast_to([1, 2, TAIL_M]))
            pyrs.append(pt)

        # matmul column splits: (psum tiles of <=2 banks, drains merged)
        # L0: 15 matmuls of 432 (4 rows); drains of 8 rows (864) x7 + 4 rows
        L0PAIRS = [(r, min(8, 60 - r)) for r in range(0, 60, 8)]
        L1PAIRS = [(0, 18, (9, 9)), (18, 12, (9, 3))]

        for c in range(NCH):
            NV = CH if c < NCH - 1 else NQ - (NCH - 1) * CH   # 42 on last chunk
            NVW = min(NV + 1, CH)   # write one extra row: absorbs row strays
            bands = [[gpool.tile([128, max(8 * PW[l], 256)], BF16,
                                 name=f"band{d}{l}",
                                 tag=f"band{d}{l}") for l in range(NL)]
                     for d in range(2)]

            for d in range(2):
                lhsT = (f0qs if d == 0 else f1qs)[:, c * CH:c * CH + NV]
                rhs = f1s if d == 0 else f0s
                gl = gf if d == 0 else gb
                pyr = pyrs[(c * 2 + d) % 6]

                t = lt[(c * 2 + d) % 2]
                t0v = t[:, 0:LSZ[0]].rearrange("p (h w) -> p h w", h=HL[0])

                # level 0: pairs of bank-aligned 432-col matmuls (8 rows)
                ei = c * 2 + d
                for r0 in range(0, 60, 8):
                    nb = min(2, (60 - r0 + 3) // 4)
                    nr = min(8, 60 - r0)
                    ps = ppool.tile([128, 1024], F32, name="ps0", tag="ps")
                    for bi in range(nb):
                        rb = r0 + bi * 4
                        nc.tensor.matmul(
                            out=ps[:NV, bi * 512:bi * 512 + 432], lhsT=lhsT,
                            rhs=rhs[:, rb * WL[0]:(rb + 4) * WL[0]],
                            start=True, stop=True)
                    dst = t0v[:NV, r0:r0 + nr, 0:WL[0]].rearrange(
                        "p (b h) w -> p b h w", b=nb)
                    src = ps[:NV].rearrange("p (b x) -> p b x", b=2)[
                        :, :, 0:432].rearrange("p b (h w) -> p b h w", h=4)[
                        :, 0:nb]
                    if ei % 5 in (0, 2):
                        nc.vector.tensor_scalar(
                            out=dst, in0=src, scalar1=INV_SQRT_D, scalar2=None,
                            op0=OP.mult)
                    else:
                        nc.scalar.activation(
                            out=dst, in_=src,
                            func=mybir.ActivationFunctionType.Copy,
                            scale=INV_SQRT_D)
                    ei += 1

                # level 1 (2 psum tiles, paired matmuls)
                t1v = t[:, LOFF[1]:LOFF[1] + LSZ[1]].rearrange(
                    "p (h w) -> p h w", h=HL[1])
                sc1 = INV_SQRT_D * 0.25
                for r0, nr, parts in L1PAIRS:
                    ps = ppool.tile([128, 1024], F32, name="ps1", tag="ps")
                    cc = 0
                    for pi, pn in enumerate(parts):
                        w = pn * WL[1]
                        nc.tensor.matmul(
                            out=ps[:NV, pi * 512:pi * 512 + w], lhsT=lhsT,
                            rhs=gl[0][:, r0 * WL[1] + cc:r0 * WL[1] + cc + w],
                            start=True, stop=True)
                        cc += w
                    cc = 0
                    for pi, pn in enumerate(parts):
                        dst = t1v[:NV, r0 + cc // WL[1]:r0 + cc // WL[1] + pn,
                                  0:WL[1]]
                        src = ps[:NV, pi * 512:pi * 512 + pn * WL[1]].rearrange(
                            "p (h w) -> p h w", h=pn)
                        if ei % 5 in (0, 2):
                            nc.vector.tensor_scalar(
                                out=dst, in0=src, scalar1=sc1, scalar2=None,
                                op0=OP.mult)
                        else:
                            nc.scalar.activation(
                                out=dst, in_=src,
                                func=mybir.ActivationFunctionType.Copy, scale=sc1)
                        ei += 1
                        cc += pn * WL[1]

                # levels 2+3 share one psum tile, separate drains
                ps23 = ppool.tile([128, LSZ[2] // PW[2] * WL[2] +
                                   LSZ[3] // PW[3] * WL[3]], F32,
                                  name="ps23", tag="ps")
                n2 = HL[2] * WL[2]
                n3 = HL[3] * WL[3]
                nc.tensor.matmul(out=ps23[:NV, 0:n2], lhsT=lhsT, rhs=gl[1][:],
                                 start=True, stop=True)
                nc.tensor.matmul(out=ps23[:NV, n2:n2 + n3], lhsT=lhsT, rhs=gl[2][:],
                                 start=True, stop=True)
                for li, off, nn in ((2, 0, n2), (3, n2, n3)):
                    tv = t[:, LOFF[li]:LOFF[li] + LSZ[li]].rearrange(
                        "p (h w) -> p h w", h=HL[li])
                    scale = INV_SQRT_D * (0.25 ** li)
                    dst = tv[:NV, :, 0:WL[li]]
                    src = ps23[:NV, off:off + nn].rearrange(
                        "p (h w) -> p h w", h=HL[li])
                    if ei % 5 in (0, 2):
                        nc.vector.tensor_scalar(
                            out=dst, in0=src, scalar1=scale, scalar2=None,
                            op0=OP.mult)
                    else:
                        nc.scalar.activation(
                            out=dst, in_=src,
                            func=mybir.ActivationFunctionType.Copy, scale=scale)
                    ei += 1

                # one pyramid write (margins pre-zeroed once per buffer)
                nc.sync.dma_start(
                    out=pyr[HEAD_M:HEAD_M + NVW * QSTRIDE, 0].rearrange(
                        "(p f) -> p f", p=NVW),
                    in_=t[:NVW])

                # band gathers: one contiguous run per query per level
                for l in range(NL):
                    run = max(7 * PW[l] + 8, 256)   # >=512B: full-rate SDMA
                    nc.gpsimd.indirect_dma_start(
                        out=bands[d][l][:NV, 0:run],
                        out_offset=None,
                        in_=pyr[:],
                        in_offset=IndirectOffsetOnAxis(
                            ap=offs[:NV, (c * 2 + d) * NL + l:
                                    (c * 2 + d) * NL + l + 1], axis=0),
                    )

            # ---------- mask + bilinear blend ----------
            pm = bpool.tile([128, 64, 8], F32, name="pm")
            pmv = pm[:NV].rearrange("p (g a) b -> p g a b", g=8)
            for d in range(2):
                for l in range(NL):
                    g = d * 4 + l
                    mcol = ((c * 2 + d) * 4 + l) * 64   # (ch,d,l,a,b) layout
                    bv = bands[d][l][:NV, 0:8 * PW[l]].rearrange(
                        "p (a w) -> p a w", a=8)
                    nc.vector.tensor_tensor(
                        out=pmv[:, g],
                        in0=bv[:, :, 0:8],
                        in1=cmb[:NV, mcol:mcol + 64].rearrange(
                            "p (a b) -> p a b", a=8),
                        op=OP.mult)

            d1 = bpool.tile([128, 64, 7], F32, name="d1")
            px = bpool.tile([128, 64, 7], F32, name="px")
            nc.vector.tensor_tensor(
                out=d1[:NV], in0=pm[:NV, :, 1:8], in1=pm[:NV, :, 0:7],
                op=OP.subtract)
            # wxe slice for this chunk: cols [c*64, (c+1)*64) in (ch,d,l,a)
            nc.vector.tensor_tensor(
                out=d1[:NV], in0=d1[:NV],
                in1=wxe[:NV, c * 64:(c + 1) * 64].unsqueeze(2).broadcast_to(
                    [NV, 64, 7]),
                op=OP.mult)
            nc.vector.tensor_tensor(
                out=px[:NV], in0=d1[:NV], in1=pm[:NV, :, 0:7], op=OP.add)

            pxv = px[:NV].rearrange("p (g a) b -> p g a b", g=8)
            e1 = bpool.tile([128, 8, 7, 7], F32, name="e1")
            ot = bpool.tile([128, 392], F32, name="ot")
            otv = ot[:NV].rearrange("p (g i j) -> p g i j", g=8, i=7)
            nc.vector.tensor_tensor(
                out=e1[:NV], in0=pxv[:, :, 1:8, :], in1=pxv[:, :, 0:7, :],
                op=OP.subtract)
            nc.vector.tensor_tensor(
                out=e1[:NV], in0=e1[:NV],
                in1=wy[:NV, c * 8:(c + 1) * 8].unsqueeze(2).unsqueeze(
                    3).broadcast_to([NV, 8, 7, 7]),
                op=OP.mult)
            nc.vector.tensor_tensor(
                out=otv, in0=e1[:NV], in1=pxv[:, :, 0:7, :], op=OP.add)

            nc.sync.dma_start(out=out[c * CH:c * CH + NV, :], in_=ot[:NV])

        for _f in reversed(_frees):
            _f()
        for p in (bpool, gpool, ppool, dpool, cpool):
            p.release()

    nc.finalize()
    return nc


def host_prepare(fmap0, fmap1, flow0, flow1, embt):
    """Build per-core input maps."""
    import ml_dtypes
    f0 = np.ascontiguousarray(fmap0.reshape(D, N).astype(ml_dtypes.bfloat16))
    f1 = np.ascontiguousarray(fmap1.reshape(D, N).astype(ml_dtypes.bfloat16))
    fl0 = flow0.reshape(2, N).astype(np.float32)
    fl1 = flow1.reshape(2, N).astype(np.float32)
    ev = float(np.asarray(embt).reshape(-1)[0])
    cst = host_consts()

    qg = np.arange(N)
    wq = (qg % W).astype(np.float32)
    hq = (qg // W).astype(np.float32)

    in_maps = []
    for core in range(NCORES):
        s = core * NQ
        sl = np.arange(s, s + NQ)
        fq0 = np.zeros((D, NQP), ml_dtypes.bfloat16)
        fq1 = np.zeros((D, NQP), ml_dtypes.bfloat16)
        fq0[:, :NQ] = f0[:, sl]
        fq1[:, :NQ] = f1[:, sl]
        aux = np.zeros((128, AUXW), np.float32)
        for ch in range(NCH):
            lo = ch * CH
            n = min(CH, NQ - lo)
            if n <= 0:
                continue
            rows = np.arange(n)
            q = s + lo + rows
            # flows: cols (c, ch, d): c*14 + ch*2 + d; d=0 -> flow1, d=1 -> flow0
            aux[rows, 0 * 14 + ch * 2 + 0] = fl1[0, q]
            aux[rows, 0 * 14 + ch * 2 + 1] = fl0[0, q]
            aux[rows, 1 * 14 + ch * 2 + 0] = fl1[1, q]
            aux[rows, 1 * 14 + ch * 2 + 1] = fl0[1, q]
            # w/h: cols 28 + c*7 + ch
            aux[rows, 28 + ch] = wq[q]
            aux[rows, 35 + ch] = hq[q]
        aux[:, 42] = ev
        in_maps.append({
            "fmap0": f0, "fmap1": f1, "f0q": fq0, "f1q": fq1,
            "aux": aux, "cst": cst,
        })
    return in_maps


_CACHED = {}


def kernel(fmap0, fmap1, flow0, flow1, embt):
    from concourse.bass_utils import run_bass_kernel_spmd

    if "nc" not in _CACHED:
        _CACHED["nc"] = build_nc()
    nc = _CACHED["nc"]

    in_maps = host_prepare(fmap0, fmap1, flow0, flow1, embt)
    res = run_bass_kernel_spmd(nc, in_maps, core_ids=list(range(NCORES)))
    outs = [r["out"][:NQ] for r in res.results]        # [810, 392] each
    corr = np.concatenate(outs, axis=0)                # [6480, 392]
    corr = corr.T.reshape(1, 392, H, W).astype(np.float32)
    full = np.concatenate(
        [corr, flow0.astype(np.float32), flow1.astype(np.float32)], axis=1)
    return full

